# revision 24
# baseline (speedup 1.0000x reference)
"""DeepAttnMISL segment-reduce kernel for 8 TRN2 NeuronCores.

Strategy: shard the N=200000 patches across 8 cores. The big matmul
phi = relu(X @ W_phi.T + b) is DMA/PE-bound, so X ships as fp8e4m3
(25.6MB/core vs 102.4MB fp32) and the PE runs fp8 DoubleRow matmuls
(2 k-planes per instruction, 2 fp8 MACs/cell/cycle) with W_phi
stationary. W_phi is scaled by 128 on the host so its values sit in
e4m3's normal range (relu is positively homogeneous, so the scale
divides back out of the sums).

Segment reduction: the host sorts patches by cluster_id and zero-pads
each cluster to a multiple of F=512 so every 512-patch block is
cluster-pure. Per block the two psum halves [128 hid, 512 patch] are
reduced to single accumulator columns on separate engines so neither
gates PSUM recycling:
  h=0: DVE tensor_scalar (add bias, max 0) with fused accum_out.
  h=1: ScalarE relu+bias activation with fused accum_out.
The [128, 2*NB] accumulator is DMA'd out in two pieces; the host folds
block sums into per-cluster sums (exact fp32), applies the zero-row
padding correction, and runs the tiny attention head in fp32.

Front-loading: the ~7us framework preamble means DMA triggers can't
fire before then, so the first 4 blocks are split in kp-halves across
the sync and scalar HW DGE queues (wt is split the same way) to
minimize time-to-first-matmul, while the PE warms the HAM clock gate
with dummy matmuls on a memset tile (no dependency on wt). Steady
state streams 2-block chunks alternating between the gpsimd (SWDGE)
and sync (HWDGE) rings.

Quantization error: ~3e-5 final rel err measured (gate 2e-2) -- fp8
noise averages out in the ~20000-patch cluster means.
"""

import numpy as np
import ml_dtypes

import concourse.mybir as mybir
import concourse.tile as tile
from concourse import bacc
from concourse.bass_utils import run_bass_kernel_spmd

N = 200000
D_IN = 1024
D_HID = 256
NUM_CLUSTERS = 10
NCORES = 8
P = 128
KSUB = D_IN // P        # 8 k-subtiles of 128
F = 512                 # patches per block (one PSUM bank in fp32)
NB = 50                 # blocks per core (sum_c ceil(n_c/512) <= 400 always)
NPC = NB * F            # 25600 padded patches per core
NBG = NB * NCORES       # 400 global blocks
W_WARM = 14             # HAM warm-up matmuls (dummy weights)
NSINGLE = 6             # leading blocks processed as G=1 groups
ACC_SPLIT = 40          # blocks whose acc columns ship early
SLOW_BLK = NB - 1       # block carried by the slow scalar DGE queue
WSCALE = 128.0          # host-side W/b scale so W lands in e4m3 normals

F8NP = ml_dtypes.float8_e4m3   # matches mybir.dt.float8e4

_CACHE = {}


def _build():
    if "nc" in _CACHE:
        return _CACHE["nc"]
    f32 = mybir.dt.float32
    f8 = mybir.dt.float8e4
    bf16 = mybir.dt.bfloat16
    DR = mybir.MatmulPerfMode.DoubleRow
    Alu = mybir.AluOpType
    nc = bacc.Bacc("TRN2", target_bir_lowering=False, debug=False, num_devices=NCORES)

    BLKW = KSUB * F  # dram columns per block

    # block-major DRAM layout: each block is one contiguous 512KB region
    # (bigger descriptor runs than the 4KB/partition strides of a
    # partition-major layout, which capped the HWDGE queue's rate)
    xt_d = nc.dram_tensor("xt", [NB * P, BLKW], f8, kind="ExternalInput").ap()
    wt_d = nc.dram_tensor("wt", [P, KSUB, D_HID], f8, kind="ExternalInput").ap()
    bb_d = nc.dram_tensor("bb", [P, 2], f32, kind="ExternalInput").ap()
    acc_d = nc.dram_tensor("acc", [P, 2 * NB], f32, kind="ExternalOutput").ap()

    # Greedy block->queue assignment by simulated finish time (rates in
    # MB/us measured from traces: sync ~0.135, gpsimd ~0.155; sync
    # triggers fire ~1.3us earlier but carry wt + the b0 half first).
    BS = F * KSUB / 1e3  # MB per block (fp8)
    RS, RG = 0.100, 0.190  # measured queue rates, MB/us
    s_t = 8.2 + (0.25 + BS / 2) / RS
    g_t = 9.5 + (BS / 2) / RG
    on_sync = {}
    for j in range(1, SLOW_BLK):
        if s_t + BS / RS <= g_t + BS / RG:
            on_sync[j] = True
            s_t += BS / RS
        else:
            on_sync[j] = False
            g_t += BS / RG

    with tile.TileContext(nc) as tc:
        with (
            tc.tile_pool(name="consts", bufs=1) as cpool,
            tc.tile_pool(name="x", bufs=12) as xpool,
            tc.tile_pool(name="ps", bufs=1, space="PSUM") as ppool,
        ):
            # --- front DMA triggers ----------------------------------
            # block 0 split in kp-halves across both fast queues; wt on
            # sync right behind; the slow scalar queue (~13GB/s) gets
            # the bias plus one late block it can trickle in all run.
            wt_sb = cpool.tile([P, KSUB, D_HID], f8)
            xt0 = cpool.tile([P, KSUB, F], f8, name="xt0")
            nc.sync.dma_start(
                out=xt0[:, 0:KSUB // 2, :], in_=xt_d[0:P, 0:BLKW // 2]
            )
            # wt in kp-pair pieces so the first matmul only waits for
            # the first 64KB, not the whole 256KB transfer
            for kp in range(KSUB // 2):
                nc.sync.dma_start(
                    out=wt_sb[:, 2 * kp:2 * kp + 2, :],
                    in_=wt_d[:, 2 * kp:2 * kp + 2, :],
                )
            nc.gpsimd.dma_start(
                out=xt0[:, KSUB // 2:KSUB, :], in_=xt_d[0:P, BLKW // 2:BLKW]
            )
            bb_sb = cpool.tile([P, 2], f32)
            nc.scalar.dma_start(out=bb_sb, in_=bb_d)
            # slow scalar queue (~7-13GB/s) only gets half the last
            # block -- a full block risks missing its deadline; the
            # fast gpsimd queue carries the other half at the end.
            xt_slow = cpool.tile([P, KSUB, F], f8, name="xtslow")
            nc.scalar.dma_start(
                out=xt_slow[:, 0:KSUB // 2, :],
                in_=xt_d[SLOW_BLK * P:(SLOW_BLK + 1) * P, 0:BLKW // 2],
            )

            # warm-up weights: zeroed on DVE so the PE has no DMA dep
            dummy_w = cpool.tile([P, 2, D_HID], f8)
            nc.vector.memset(dummy_w, 0)
            # throwaway main-out targets for the fused reductions
            dummy_v = cpool.tile([P, F], bf16)
            dummy_s = cpool.tile([P, F], bf16)
            zeros_v = cpool.tile([P, F], f32)
            nc.vector.memset(zeros_v, 0)
            acc_sb = cpool.tile([P, 2 * NB], f32)

            # per-block triggers, interleaved so arrival order tracks
            # consumption order; buffer-free waits on the rotating pool
            # act as a prefetch pipeline (both engines are idle anyway).
            xt = {0: xt0, SLOW_BLK: xt_slow}
            for j in range(1, SLOW_BLK):
                t = xpool.tile([P, KSUB, F], f8, tag="xt", name="xt_sb")
                xt[j] = t
                eng = nc.sync if on_sync[j] else nc.gpsimd
                eng.dma_start(out=t, in_=xt_d[j * P:(j + 1) * P, :])
            nc.gpsimd.dma_start(
                out=xt_slow[:, KSUB // 2:KSUB, :],
                in_=xt_d[SLOW_BLK * P:(SLOW_BLK + 1) * P, BLKW // 2:BLKW],
            )

            # --- HAM clock warm-up: dummy matmuls, no input deps ------
            warm_ps = ppool.tile([P, F], f32, tag="ps00", name="warm_ps")
            for _ in range(W_WARM):
                nc.tensor.matmul(
                    warm_ps[:, 0:D_HID],
                    dummy_w[:, 0:2, 0:P],
                    dummy_w,
                    start=True,
                    stop=True,
                    perf_mode=DR,
                )

            def block_reduce(ps, blk):
                """Fused relu+bias+sum of both psum halves for block blk."""
                nc.vector.scalar_tensor_tensor(
                    out=dummy_v,
                    in0=ps[0],
                    scalar=bb_sb[:, 0:1],
                    in1=zeros_v,
                    op0=Alu.add,
                    op1=Alu.max,
                    accum_out=acc_sb[:, 2 * blk:2 * blk + 1],
                )
                nc.scalar.activation(
                    dummy_s,
                    ps[1],
                    mybir.ActivationFunctionType.Relu,
                    bias=bb_sb[:, 1:2],
                    accum_out=acc_sb[:, 2 * blk + 1:2 * blk + 2],
                )

            # --- compute: leading singles (arrival-paced), then groups
            # of 4 blocks sharing stationaries -------------------------
            blk0 = 0
            while blk0 < NB:
                G = 1 if blk0 < NSINGLE else min(4, NB - blk0)
                ps = [
                    [
                        ppool.tile(
                            [P, F], f32,
                            tag=f"ps{h}{(blk0 + s) % 4}",
                            name=f"ps{h}{(blk0 + s) % 4}",
                        )
                        for s in range(G)
                    ]
                    for h in range(2)
                ]
                for h in range(2):
                    for kp in range(KSUB // 2):
                        w_ap = wt_sb[:, 2 * kp:2 * kp + 2, h * P:(h + 1) * P]
                        for s in range(G):
                            nc.tensor.matmul(
                                ps[h][s],
                                w_ap,
                                xt[blk0 + s][:, 2 * kp:2 * kp + 2, :],
                                start=(kp == 0),
                                stop=(kp == KSUB // 2 - 1),
                                perf_mode=DR,
                            )
                for s in range(G):
                    blk = blk0 + s
                    block_reduce([ps[0][s], ps[1][s]], blk)
                    if blk == ACC_SPLIT - 1:
                        nc.scalar.dma_start(
                            out=acc_d[:, 0:2 * ACC_SPLIT],
                            in_=acc_sb[:, 0:2 * ACC_SPLIT],
                        )
                blk0 += G

            nc.gpsimd.dma_start(
                out=acc_d[:, 2 * ACC_SPLIT:2 * NB],
                in_=acc_sb[:, 2 * ACC_SPLIT:2 * NB],
            )

    nc.compile()
    _CACHE["nc"] = nc
    return nc


def _prepare_in_maps(X, cluster_id, W_phi, b_phi):
    cid = np.asarray(cluster_id).astype(np.int64)
    x2 = np.asarray(X, np.float32).reshape(-1, D_IN)

    order = np.argsort(cid, kind="stable")
    counts = np.bincount(cid, minlength=NUM_CLUSTERS)

    # Cluster-pure 512-patch blocks: sorted patches, each cluster padded
    # with the zero row (index N) to a multiple of F.
    idx = np.full(NBG * F, N, dtype=np.int64)
    block_cluster = np.zeros(NBG, dtype=np.int64)
    pad_per_cluster = np.zeros(NUM_CLUSTERS, dtype=np.int64)
    b = 0
    off = 0
    for cc in range(NUM_CLUSTERS):
        n_c = int(counts[cc])
        nb_c = -(-n_c // F)
        idx[b * F:b * F + n_c] = order[off:off + n_c]
        block_cluster[b:b + nb_c] = cc
        pad_per_cluster[cc] = nb_c * F - n_c
        b += nb_c
        off += n_c
    block_cluster[b:] = 0
    pad_per_cluster[0] += (NBG - b) * F

    Xq = np.empty((N + 1, D_IN), dtype=F8NP)
    Xq[:N] = x2.astype(F8NP)
    Xq[N] = 0

    wp = np.asarray(W_phi, np.float32) * WSCALE          # [256, 1024]
    # wt[p, jj, m] = WSCALE * W_phi[m, jj*128 + p]
    wt = np.ascontiguousarray(
        wp.T.reshape(KSUB, P, D_HID).transpose(1, 0, 2)
    ).astype(F8NP)
    bvec = np.asarray(b_phi, np.float32) * WSCALE
    bbias = np.empty((P, 2), np.float32)
    bbias[:, 0] = bvec[:P]
    bbias[:, 1] = bvec[P:]
    in_maps = []
    for core in range(NCORES):
        rows = idx[core * NPC:(core + 1) * NPC]
        xr = Xq[rows]                                    # [NPC, 1024] fp8
        # xdev[b*128+p, jj*F+n] = X[row(b*F+n), jj*128+p]  (block-major)
        xdev = np.ascontiguousarray(
            xr.reshape(NB, F, KSUB, P).transpose(0, 3, 2, 1)
        ).reshape(NB * P, KSUB * F)
        in_maps.append({"xt": xdev, "wt": wt, "bb": bbias})

    meta = (block_cluster, pad_per_cluster, counts)
    return in_maps, meta


def kernel(X, cluster_id, W_phi, b_phi, W1, b1, Wa, ba, Wb, bb, Wc, bc, Wo, bo):
    in_maps, (block_cluster, pad_per_cluster, counts_i) = _prepare_in_maps(
        X, cluster_id, W_phi, b_phi
    )

    nc = _build()
    res = run_bass_kernel_spmd(nc, in_maps, list(range(NCORES)))

    blocksums = np.empty((NBG, D_HID), np.float32)
    for core in range(NCORES):
        a = np.asarray(res.results[core]["acc"], np.float32).reshape(P, NB, 2)
        # blocksums[core*NB + b, h*128 + p] = a[p, b, h]
        blocksums[core * NB:(core + 1) * NB] = a.transpose(1, 2, 0).reshape(NB, D_HID)

    sums = np.zeros((NUM_CLUSTERS, D_HID), np.float32)
    np.add.at(sums, block_cluster, blocksums)
    sums /= WSCALE
    # padding rows contribute relu(0 @ W + b) = relu(b_phi) each
    relu_b = np.maximum(np.asarray(b_phi, np.float32), 0.0)
    sums -= pad_per_cluster[:, None].astype(np.float32) * relu_b[None, :]

    counts = counts_i.astype(np.float32)

    # tiny attention-pooling + output head, fp32 on host (matches reference)
    h = np.where(counts[:, None] > 0, sums / np.maximum(counts, 1.0)[:, None], 0.0).astype(np.float32)
    h1 = np.maximum(h @ np.asarray(W1, np.float32).T + b1, 0.0).astype(np.float32)
    a = np.tanh(h1 @ np.asarray(Wa, np.float32).T + ba).astype(np.float32)
    g = (1.0 / (1.0 + np.exp(-(h1 @ np.asarray(Wb, np.float32).T + bb)))).astype(np.float32)
    scores = ((a * g) @ np.asarray(Wc, np.float32).T + bc).astype(np.float32)  # [10, 1]
    s = scores.T  # [1, 10]
    e = np.exp(s - s.max(axis=-1, keepdims=True))
    A = (e / e.sum(axis=-1, keepdims=True)).astype(np.float32)
    H = (A @ h1).astype(np.float32)
    out = (H @ np.asarray(Wo, np.float32).T + bo).astype(np.float32)
    return out
